# revision 15
# baseline (speedup 1.0000x reference)
"""ANFIS forward kernel for 8 TRN2 NeuronCores (data-parallel over batch).

Reference (per batch element, I=2 inputs, M=2 MFs, R=4 rules):
    mu[i,m] = 1 / (1 + |(x_i - c_im)/a_im|^(2 b_im))
    w_r     = mu[0,m] * mu[1,n]          (r = 2m + n)
    out     = sum_r (w_r / sum w) * (W[r]. x + Bd[r])

Division-free form with q_im = ((x_i-c_im)/a_im)^(2 b_im), d = 1 + q:

    out = [ d01*(d11*g0 + d10*g1) + d00*(d11*g2 + d10*g3) ]
          / [ (d00+d01) * (d10+d11) ]
    g_r = W[r,0]*x0 + W[r,1]*x1 + Bd[r]

Engine split per [128, FC] chunk:
  ScalarE : 8 Square LUT ops -> q_im (fp16 out)
  TensorE : g_r = diag(W[r,0])@x0 + diag(W[r,1])@x1 into PSUM (fp32)
  GpSimd  : S0 = q00+q01+2, S1 = q10+q11+2 (fused STT, fp32)
  VectorE : custom op m_r = (q+1)*((g_r+Bd_r)/256) [fp16], adds/STTs for
            the numerator, D = S0*S1, fast approx reciprocal, final
            (N*256)*(1/D) -> fp32 out.
The fp16 mid-chain is scaled by 1/256 to stay in range (validated on
host: rel err ~4.5e-4 vs the fp32 reference).
"""

import numpy as np
from contextlib import ExitStack

import concourse.bass as bass
import concourse.bacc as bacc
import concourse.tile as tile
from concourse import mybir
from concourse.bass_utils import run_bass_kernel_spmd
from concourse import dve_ops
from concourse.dve_spec import Spec, Src0, Src1, C0, C1, C2, One, lower, _has_src1
from concourse.dve_uop import DveOpSpec

N_CORES = 8
I_FEAT = 2
M_MF = 2
R_RULES = 4
N_TOTAL = 4_194_304
NC = N_TOTAL // N_CORES          # 524288 elements per core
P = 128
F_TOT = NC // P                  # 4096 per partition
FC = 1024                        # max chunk free-dim (PSUM tag sizing)
HALF = 512                       # matmul moving free-dim / PSUM bank
# ramp-in schedule: small first chunks so compute starts early
CHUNKS = [256, 256, 512, 1024, 1024, 1024]
assert sum(CHUNKS) == F_TOT

F32 = mybir.dt.float32
F16 = mybir.dt.float16
BF16 = mybir.dt.bfloat16
ALU = mybir.AluOpType
ACTF = mybir.ActivationFunctionType
MSCALE = 1.0 / 256.0             # fp16 range headroom for the numerator


def _register_op(name, spec):
    for op in dve_ops.OPS:
        if op.name == name:
            return op
    row = dve_ops._CUSTOM_DVE_ROW_BASE + len(dve_ops.OPS)
    shas = {
        ver: DveOpSpec(name=name, opcode=row, uops=lower(spec, ver=ver),
                       rd1_en=_has_src1(spec)).sha(ver)
        for ver in ("v3", "v4")
    }
    op = dve_ops.DveOp(name, spec, subdim=False, uops_sha=shas)
    dve_ops._SUB_OPCODE_FOR_NAME[name] = row
    dve_ops.OPS.append(op)
    dve_ops.CUSTOM_DVE_SPECS[name] = spec
    return op


# out = (in0 + 1) * ((in1 + s0) * imm2) — the d*(g+Bd) product
DG_OP = _register_op("ANFIS_DG", Spec(
    body=(Src0 + One) * ((Src1 + C0) * C2),
    reference=lambda in0, in1, s0, s1, imm2: (
        (in0.astype(np.float32) + 1.0) * ((in1 + s0) * imm2)
    ),
))
# out = ((in0 + s0) * (in1 + s1)) * imm2 — scaled denominator
DD_OP = _register_op("ANFIS_DDS", Spec(
    body=((Src0 + C0) * (Src1 + C1)) * C2,
    reference=lambda in0, in1, s0, s1, imm2: (
        ((in0.astype(np.float32) + s0) * (in1 + s1)) * imm2
    ),
))


def _make_wdiag(W):
    """[128, 8*128] fp16: block d=2r+i holds diag(W[r,i])."""
    wd = np.zeros((P, 2 * R_RULES * P), np.float32)
    for r in range(R_RULES):
        for i in range(I_FEAT):
            d = 2 * r + i
            wd[np.arange(P), d * P + np.arange(P)] = W[r, i]
    return wd.astype(np.float16)


def _build(a, b, c, W, Bd):
    nc = bacc.Bacc("TRN2", num_devices=N_CORES)
    x_d = nc.dram_tensor("x", [I_FEAT, NC], F32, kind="ExternalInput")
    wd_d = nc.dram_tensor("wd", [P, 2 * R_RULES * P], F16, kind="ExternalInput")
    o_d = nc.dram_tensor("out", [NC], F32, kind="ExternalOutput")

    x0v = x_d.ap()[0].rearrange("(p f) -> p f", p=P)
    x1v = x_d.ap()[1].rearrange("(p f) -> p f", p=P)
    ov = o_d.ap().rearrange("(p f) -> p f", p=P)

    with tile.TileContext(nc) as tc, ExitStack() as ctx:
        io = ctx.enter_context(tc.tile_pool(name="io", bufs=3))
        tp = ctx.enter_context(tc.tile_pool(name="tp", bufs=2))
        sp = ctx.enter_context(tc.tile_pool(name="sp", bufs=4))
        cpool = ctx.enter_context(tc.tile_pool(name="const", bufs=1))
        psum = ctx.enter_context(tc.tile_pool(name="psum", bufs=1, space="PSUM"))

        # constants: ACT bias columns + the 8 diagonal stationaries
        bias_vals = {float(-c[i, m] / a[i, m]) for i in range(I_FEAT)
                     for m in range(M_MF)}
        bias_ap = {}
        for v in sorted(bias_vals):
            t = cpool.tile([P, 1], F32, tag=f"cst{v}")
            nc.gpsimd.memset(t[:], v)
            bias_ap[v] = t
        wd = cpool.tile([P, 2 * R_RULES * P], F16, tag="wd")
        nc.sync.dma_start(out=wd[:], in_=wd_d.ap())

        coff = 0
        for ci, fc in enumerate(CHUNKS):
            sl = bass.ds(coff, fc)
            coff += fc

            x0 = io.tile([P, fc], F32, tag="x0")
            nc.sync.dma_start(out=x0[:], in_=x0v[:, sl])
            x1 = io.tile([P, fc], F32, tag="x1")
            nc.sync.dma_start(out=x1[:], in_=x1v[:, sl])

            # ScalarE: fp16 copies of x for the TensorE moving operand
            # (NOT GpSimd: its SBUF-port lock starves VectorE 2x-mode ops)
            x0b = tp.tile([P, fc], F16, tag="x0b")
            nc.scalar.copy(x0b[:], x0[:])
            x1b = tp.tile([P, fc], F16, tag="x1b")
            nc.scalar.copy(x1b[:], x1[:])

            # ScalarE: q_im = ((x_i-c_im)/a_im)^(2 b_im), fp16 out
            q = {}
            for i, xi in ((0, x0), (1, x1)):
                for m in range(M_MF):
                    scale = float(1.0 / a[i, m])
                    bias = float(-c[i, m] / a[i, m])
                    s_im = sp.tile([P, fc], F32, tag="s")
                    bias_arg = bias_ap[bias][:] if bias != 0.0 else 0.0
                    nc.scalar.activation(s_im[:], xi[:], ACTF.Square,
                                         bias=bias_arg, scale=scale)
                    q_im = tp.tile([P, fc], F16, tag=f"q{i}{m}")
                    if abs(float(b[i, m]) - 2.0) < 1e-7:
                        nc.scalar.activation(q_im[:], s_im[:], ACTF.Square)
                    else:
                        nc.scalar.activation(s_im[:], s_im[:], ACTF.Ln)
                        nc.scalar.activation(q_im[:], s_im[:], ACTF.Exp,
                                             scale=float(b[i, m]))
                    q[i, m] = q_im

            # TensorE: g_r = diag(W[r,0])@x0 + diag(W[r,1])@x1 (PSUM, fp32)
            g = []
            for r in range(R_RULES):
                g_r = psum.tile([P, fc], F32, tag=f"g{r}")
                for i, xi in ((0, x0b), (1, x1b)):
                    lhsT = wd[:, (2 * r + i) * P:(2 * r + i + 1) * P]
                    for h0 in range(0, fc, HALF):
                        hs = bass.ds(h0, min(HALF, fc - h0))
                        nc.tensor.matmul(g_r[:, hs], lhsT, xi[:, hs],
                                         start=(i == 0), stop=(i == 1))
                g.append(g_r)

            # GpSimd: S0 = q00+q01, S1 = q10+q11 (fp32; +2 folded into DD)
            s0 = tp.tile([P, fc], F32, tag="s0")
            nc.gpsimd.tensor_add(s0[:], q[0, 0][:], q[0, 1][:])
            s1 = tp.tile([P, fc], F32, tag="s1")
            nc.gpsimd.tensor_add(s1[:], q[1, 0][:], q[1, 1][:])

            # VectorE: numerator in fp16 (scaled by 1/256)
            def dg(qt, gt, bd, tag):
                m_t = tp.tile([P, fc], F16, tag=tag)
                nc.vector._custom_dve(DG_OP, out=m_t[:], in0=qt[:], in1=gt[:],
                                      s0=float(bd), imm2=MSCALE)
                return m_t

            m1 = dg(q[1, 1], g[0], Bd[0], "m1")
            m2 = dg(q[1, 0], g[1], Bd[1], "m2")
            u = tp.tile([P, fc], F16, tag="u")
            nc.vector.tensor_add(u[:], m1[:], m2[:])
            m3 = dg(q[1, 1], g[2], Bd[2], "m3")
            m4 = dg(q[1, 0], g[3], Bd[3], "m4")
            w = tp.tile([P, fc], F16, tag="w")
            nc.vector.tensor_add(w[:], m3[:], m4[:])
            d01 = tp.tile([P, fc], F16, tag="d01")
            nc.vector.tensor_scalar_add(d01[:], q[0, 1][:], 1.0)
            n1 = tp.tile([P, fc], F16, tag="n1")
            nc.vector.tensor_mul(n1[:], d01[:], u[:])
            d00 = tp.tile([P, fc], F16, tag="d00")
            nc.vector.tensor_scalar_add(d00[:], q[0, 0][:], 1.0)
            n2 = tp.tile([P, fc], F16, tag="n2")
            nc.vector.tensor_mul(n2[:], d00[:], w[:])
            nt = tp.tile([P, fc], F16, tag="nt")
            nc.vector.tensor_add(nt[:], n1[:], n2[:])

            # VectorE tail: out = N * approx(1/(((S0+2)*(S1+2))*MSCALE))
            d_t = tp.tile([P, fc], F32, tag="d")
            nc.vector._custom_dve(DD_OP, out=d_t[:], in0=s0[:], in1=s1[:],
                                  s0=2.0, s1=2.0, imm2=MSCALE)
            rd = tp.tile([P, fc], F32, tag="rd")
            nc.vector.reciprocal_approx_fast(rd[:], d_t[:])
            o = io.tile([P, fc], F32, tag="o")
            nc.vector.tensor_mul(o[:], nt[:], rd[:])
            nc.sync.dma_start(out=ov[:, sl], in_=o[:])

    nc.compile()
    return nc


_CACHE = {}


def _get_built(a, b, c, W, Bd):
    key = (a.tobytes(), b.tobytes(), c.tobytes(), W.tobytes(), Bd.tobytes())
    if key not in _CACHE:
        _CACHE[key] = (_build(a, b, c, W, Bd), _make_wdiag(W))
    return _CACHE[key]


def run(x, a, b, c, W, Bd, trace=False):
    nc, wd = _get_built(np.asarray(a), np.asarray(b), np.asarray(c),
                        np.asarray(W), np.asarray(Bd))
    x = np.ascontiguousarray(np.asarray(x, dtype=np.float32))
    in_maps = [{"x": np.ascontiguousarray(x[:, i * NC:(i + 1) * NC]), "wd": wd}
               for i in range(N_CORES)]
    res = run_bass_kernel_spmd(nc, in_maps, list(range(N_CORES)), trace=trace)
    out = np.concatenate([res.results[i]["out"] for i in range(N_CORES)])
    return out.astype(np.float32), res


def kernel(x, a, b, c, W, Bd):
    out, _ = run(x, a, b, c, W, Bd, trace=False)
    return out
